# revision 29
# baseline (speedup 1.0000x reference)
"""Deformable warp (bilinear grid_sample with shared displacement field) on 8 trn2 cores.

Problem: source [8,16,512,512] f32, displacement [1,2,512,512] f32 (shared over batch).
out[b,c,y,x] = bilinear_sample(source[b,c], x + dx[y,x]*255.5, y + dy[y,x]*255.5),
align_corners=True, zero padding.

Strategy:
  - Host re-layouts source to a channel-last "Z slab": zslab[p] = (px[p], px[p+512])
    where px[p] is pixel p's 128 (b,c) values (512B). One gathered run of 512 f32
    (2KB) starting at zslab[y0*512+x0] therefore contains all four bilinear corners
    (y0/y1 x x0/x1) for one output pixel. The slab is replicated to every core
    (host->HBM upload is not part of HW exec time).
  - Spatial sharding: core q computes output rows [64q, 64q+64) for ALL batches
    and channels. No cross-core communication.
  - Per core: DVE computes sampling coords/weights/indices from the displacement
    rows; GPSIMD indirect DMAs gather one 2KB run per output pixel (one index per
    partition per call - the only indirect-DMA shape the HW DGE supports); DVE
    multiplies by the 4 corner weights (broadcast over the 128 (b,c) lanes along a
    stride-0 AP dim) and reduces; results DMA back to HBM as [pixel, 128] which
    the host transposes back to [B,C,H,W].

Zero-padding semantics are realized by clamping the fetch base into the slab and
zeroing the weights of out-of-image corners (the slab has generous zero padding
so every clamped fetch is in-bounds and finite).
"""

import sys

sys.path.insert(0, "/opt/trn_rl_repo")

import numpy as np

import concourse.bass as bass
import concourse.bacc as bacc
import concourse.mybir as mybir
import concourse.tile as tile

F32 = mybir.dt.float32
I32 = mybir.dt.int32

B, C, H, W = 8, 16, 512, 512
BC = B * C  # 128
NCORES = 8
ROWS = H // NCORES  # 64 output rows per core
NPX = ROWS * W  # 32768 pixels per core
CHUNKS = NPX // 128  # 256 chunks of 128 pixels
TILE_CHUNKS = 16  # chunks per pipeline tile (2048 px)
NTILES = CHUNKS // TILE_CHUNKS  # 16

# Z-slab geometry: ext = [FRONT zero rows][H*W pixel rows][BACK zero rows], each
# row 128 f32.  zslab[i] = ext[i] | ext[i+512], i in [0, len(ext)-512).
# Fetch base for a pixel: i = (y0m*512 + xb) + FRONT with y0m,xb in [-1, 511],
# so min i = FRONT - 513 >= 0 -> FRONT = 513; the run reads zslab[i], zslab[i+1]
# -> ext up to i+1+512: max = FRONT + 262143 + 513 = FRONT + 262656 -> BACK = 514.
FRONT = 513
BACK = 514
NEXT = FRONT + H * W + BACK
NZ = NEXT - 512  # zslab rows

AluOp = mybir.AluOpType


def _ap(handle, offset, dims):
    return bass.AP(handle, offset, [list(d) for d in dims])


def build_bass(reps=1):
    nc = bacc.Bacc()
    nc.num_devices = NCORES

    zslab = nc.declare_dram_parameter("zslab", [NZ, 2 * BC], F32, isOutput=False)
    disp = nc.declare_dram_parameter("disp", [2, ROWS, W], F32, isOutput=False)
    tabs = nc.declare_dram_parameter("tabs", [128, 2 * CHUNKS], F32, isOutput=False)
    out = nc.declare_dram_parameter("out", [NPX, BC], F32, isOutput=True)

    with tile.TileContext(nc) as tc:
        with (
            tc.tile_pool(name="res", bufs=1) as res,
            tc.tile_pool(name="gat", bufs=2) as gat,
            tc.tile_pool(name="ot", bufs=2) as ot,
        ):
            v = nc.vector
            _tagn = [0]

            def rtile(shape, dtype):
                _tagn[0] += 1
                return res.tile(shape, dtype, tag=f"rt{_tagn[0]}", name=f"rt{_tagn[0]}")

            # ---- resident tensors -------------------------------------------------
            # pixel p (raster within this core's 64 rows) lives at
            # [partition = p % 128, chunk = p // 128]; chunk = 4*cy + cx where
            # y_local = cy, x = (p%128) + 128*cx.
            dxy = rtile([128, 2 * CHUNKS], F32)
            wts = rtile([128, CHUNKS, 4], F32)
            idx = rtile([128, CHUNKS], I32)

            # displacement load, both channels in one DMA:
            # value at (part, (ch*ROWS+cy)*4 + cx) = disp[ch, cy, part + 128*cx]
            nc.sync.dma_start(
                out=dxy[:],
                in_=_ap(disp, 0, [(1, 128), (W, 2 * ROWS), (128, 4)]),
            )
            dx = dxy[:, 0:CHUNKS]
            dy = dxy[:, CHUNKS:2 * CHUNKS]

            # per-pixel normalized-coordinate tables (host-arranged, bit-exact
            # jnp.linspace values): xs_pix | ys_pix halves
            tabt = rtile([128, 2 * CHUNKS], F32)
            nc.sync.dma_start(out=tabt[:], in_=tabs[:])
            xs_pix = tabt[:, 0:CHUNKS]
            ys_pix = tabt[:, CHUNKS:2 * CHUNKS]

            # sampling coords in pixel space, matching the reference op-for-op:
            #   g = table + d;  pix = (g + 1) * 0.5 * (size-1)
            gx = rtile([128, CHUNKS], F32)
            gy = rtile([128, CHUNKS], F32)
            v.tensor_tensor(out=gx[:], in0=xs_pix, in1=dx, op=AluOp.add)
            v.tensor_scalar(out=gx[:], in0=gx[:], scalar1=1.0, scalar2=(W - 1) / 2.0,
                            op0=AluOp.add, op1=AluOp.mult)
            v.tensor_tensor(out=gy[:], in0=ys_pix, in1=dy, op=AluOp.add)
            v.tensor_scalar(out=gy[:], in0=gy[:], scalar1=1.0, scalar2=(H - 1) / 2.0,
                            op0=AluOp.add, op1=AluOp.mult)

            def floor_frac(g, lim):
                """returns (g0 = floor(g) f32, frac, w0=1-frac, v0, v1, gb=clamp(g0,-1,lim-1))"""
                t_i = rtile([128, CHUNKS], I32)
                v.tensor_copy(out=t_i[:], in_=g[:])
                tf = rtile([128, CHUNKS], F32)
                v.tensor_copy(out=tf[:], in_=t_i[:])
                adj = rtile([128, CHUNKS], F32)
                v.tensor_tensor(out=adj[:], in0=tf[:], in1=g[:], op=AluOp.is_gt)
                g0 = rtile([128, CHUNKS], F32)
                v.tensor_tensor(out=g0[:], in0=tf[:], in1=adj[:], op=AluOp.subtract)
                fr = rtile([128, CHUNKS], F32)
                v.tensor_tensor(out=fr[:], in0=g[:], in1=g0[:], op=AluOp.subtract)
                w0 = rtile([128, CHUNKS], F32)
                v.tensor_scalar(out=w0[:], in0=fr[:], scalar1=-1.0, scalar2=1.0,
                                op0=AluOp.mult, op1=AluOp.add)
                m0 = rtile([128, CHUNKS], F32)
                m1 = rtile([128, CHUNKS], F32)
                v0 = rtile([128, CHUNKS], F32)
                v1 = rtile([128, CHUNKS], F32)
                v.tensor_scalar(out=m0[:], in0=g0[:], scalar1=0.0, scalar2=None, op0=AluOp.is_ge)
                v.tensor_scalar(out=m1[:], in0=g0[:], scalar1=float(lim - 1), scalar2=None, op0=AluOp.is_le)
                v.tensor_tensor(out=v0[:], in0=m0[:], in1=m1[:], op=AluOp.mult)
                v.tensor_scalar(out=m0[:], in0=g0[:], scalar1=-1.0, scalar2=None, op0=AluOp.is_ge)
                v.tensor_scalar(out=m1[:], in0=g0[:], scalar1=float(lim - 2), scalar2=None, op0=AluOp.is_le)
                v.tensor_tensor(out=v1[:], in0=m0[:], in1=m1[:], op=AluOp.mult)
                gb = rtile([128, CHUNKS], F32)
                v.tensor_scalar(out=gb[:], in0=g0[:], scalar1=-1.0, scalar2=float(lim - 1),
                                op0=AluOp.max, op1=AluOp.min)
                return g0, fr, w0, v0, v1, gb

            x0f, fx, wx0, vx0, vx1, xb = floor_frac(gx, W)
            y0f, fy, wy0, vy0, vy1, yb = floor_frac(gy, H)

            # gather index first (unblocks the gather pipeline):
            # (yb*512 + xb) + FRONT, all values exact in f32
            idf = rtile([128, CHUNKS], F32)
            v.scalar_tensor_tensor(out=idf[:], in0=yb[:], scalar=float(W), in1=xb[:],
                                   op0=AluOp.mult, op1=AluOp.add)
            v.tensor_scalar(out=idf[:], in0=idf[:], scalar1=float(FRONT), scalar2=None,
                            op0=AluOp.add)
            v.tensor_copy(out=idx[:], in_=idf[:])

            # masked 1-D weights
            wxa = rtile([128, CHUNKS], F32)
            wxb = rtile([128, CHUNKS], F32)
            wya = rtile([128, CHUNKS], F32)
            wyb = rtile([128, CHUNKS], F32)
            v.tensor_tensor(out=wxa[:], in0=wx0[:], in1=vx0[:], op=AluOp.mult)
            v.tensor_tensor(out=wxb[:], in0=fx[:], in1=vx1[:], op=AluOp.mult)
            v.tensor_tensor(out=wya[:], in0=wy0[:], in1=vy0[:], op=AluOp.mult)
            v.tensor_tensor(out=wyb[:], in0=fy[:], in1=vy1[:], op=AluOp.mult)

            # corner weights, gathered-run order (r0x0, r1x0, r0x1, r1x1):
            for k, (a, b) in enumerate(((wya, wxa), (wyb, wxa), (wya, wxb), (wyb, wxb))):
                wk = _ap(wts.tensor, wts[:].offset + k, [(wts[:].ap[0][0], 128), (4, CHUNKS)])
                v.tensor_tensor(out=wk, in0=a[:], in1=b[:], op=AluOp.mult)

            # ---- main pipeline ----------------------------------------------------
            import contextlib
            loop_ctx = tc.For_i(0, reps) if reps > 1 else contextlib.nullcontext()
            with loop_ctx:
                main_pipeline(nc, tc, v, zslab, out, wts, idx, gat, ot)

    return nc


def main_pipeline(nc, tc, v, zslab, out, wts, idx, gat, ot):
    if True:
        if True:
            for t in range(NTILES):
                # gathered tile: memory [part][chunk][4 corners x 128bc]
                g = gat.tile([128, TILE_CHUNKS, 4 * BC], F32)
                gp = g[:].ap[0][0]
                for c in range(TILE_CHUNKS):
                    cg = t * TILE_CHUNKS + c
                    nc.gpsimd.indirect_dma_start(
                        out=g[:, c, :],
                        out_offset=None,
                        in_=zslab[:],
                        in_offset=bass.IndirectOffsetOnAxis(ap=idx[:, cg:cg + 1], axis=0),
                    )

                # multiply by corner weights (broadcast over the 128 bc lanes)
                g_m = _ap(g.tensor, g[:].offset,
                          [(gp, 128), (4 * BC, TILE_CHUNKS), (BC, 4), (1, BC)])
                w_m = _ap(wts.tensor, wts[:].offset + t * TILE_CHUNKS * 4,
                          [(wts[:].ap[0][0], 128), (4, TILE_CHUNKS), (1, 4), (0, BC)])
                v.tensor_tensor(out=g_m, in0=g_m, in1=w_m, op=AluOp.mult)

                # reduce the 4 corners: pairwise adds (cheaper than tensor_reduce:
                # each 2-input add reads both operands in one cycle)
                h = ot.tile([128, TILE_CHUNKS, 2 * BC], F32, tag="h", name=f"h_{t}")
                ga = _ap(g.tensor, g[:].offset,
                         [(gp, 128), (4 * BC, TILE_CHUNKS), (1, 2 * BC)])
                gb2 = _ap(g.tensor, g[:].offset + 2 * BC,
                          [(gp, 128), (4 * BC, TILE_CHUNKS), (1, 2 * BC)])
                v.tensor_tensor(out=h[:], in0=ga, in1=gb2, op=AluOp.add)
                o = ot.tile([128, TILE_CHUNKS, BC], F32)
                hp = h[:].ap[0][0]
                ha = _ap(h.tensor, h[:].offset,
                         [(hp, 128), (2 * BC, TILE_CHUNKS), (1, BC)])
                hb = _ap(h.tensor, h[:].offset + BC,
                         [(hp, 128), (2 * BC, TILE_CHUNKS), (1, BC)])
                v.tensor_tensor(out=o[:], in0=ha, in1=hb, op=AluOp.add)

                # writeback: pixel p = part + 128*(t*TILE_CHUNKS + chunk) at out[p, :]
                out_t = _ap(out, t * TILE_CHUNKS * 128 * BC,
                            [(BC, 128), (128 * BC, TILE_CHUNKS), (1, BC)])
                nc.sync.dma_start(out=out_t, in_=o[:])


def linspace_tables():
    """The reference's jnp.linspace(-1, 1, size) values, bit-exact (computed on CPU)."""
    import jax

    with jax.default_device(jax.devices("cpu")[0]):
        xs = np.asarray(jax.numpy.linspace(-1.0, 1.0, W, dtype=np.float32))
        ys = np.asarray(jax.numpy.linspace(-1.0, 1.0, H, dtype=np.float32))
    return xs, ys


def coord_tables(q):
    """Per-pixel linspace tables in the kernel's [part, chunk] pixel layout, core q.
    Returns one [128, 2*CHUNKS] array: xs half | ys half."""
    xs, ys = linspace_tables()
    cx = np.arange(CHUNKS) % 4
    cy = np.arange(CHUNKS) // 4
    part = np.arange(128)
    xs_pix = xs[part[:, None] + 128 * cx[None, :]]
    ys_pix = np.broadcast_to(ys[q * ROWS + cy][None, :], (128, CHUNKS))
    return np.ascontiguousarray(np.hstack([xs_pix, ys_pix]), np.float32)


def build_zslab(source):
    """Channel-last Z slab: zslab[i] = ext[i] | ext[i+512]."""
    ext = np.zeros((NEXT, BC), np.float32)
    ext[FRONT:FRONT + H * W] = source.transpose(2, 3, 0, 1).reshape(H * W, BC)
    z = np.empty((NZ, 2 * BC), np.float32)
    z[:, :BC] = ext[:NZ]
    z[:, BC:] = ext[512:512 + NZ]
    return z


def make_in_maps(source, displacement):
    source = np.ascontiguousarray(source, dtype=np.float32)
    displacement = np.ascontiguousarray(displacement, dtype=np.float32)
    assert source.shape == (B, C, H, W)
    assert displacement.shape == (1, 2, H, W)
    z = build_zslab(source)
    d = displacement[0]
    in_maps = []
    for q in range(NCORES):
        in_maps.append({
            "zslab": z,
            "disp": np.ascontiguousarray(d[:, q * ROWS:(q + 1) * ROWS, :]),
            "tabs": coord_tables(q),
        })
    return in_maps


_NC_CACHE = None


def _get_nc():
    global _NC_CACHE
    if _NC_CACHE is None:
        _NC_CACHE = build_bass()
        if not _NC_CACHE.is_finalized():
            _NC_CACHE.finalize()
    return _NC_CACHE


def assemble_output(outs):
    full = np.concatenate([o.reshape(ROWS, W, B, C) for o in outs], axis=0)
    return np.ascontiguousarray(full.transpose(2, 3, 0, 1))


def kernel(source, displacement):
    from concourse.bass_utils import run_bass_kernel_spmd

    in_maps = make_in_maps(source, displacement)
    res = run_bass_kernel_spmd(_get_nc(), in_maps, list(range(NCORES)))
    return assemble_output([res.results[q]["out"] for q in range(NCORES)])


def measure_hw(source, displacement, reps=4097, warm=2):
    """Estimate per-invocation HW time via a device-looped program.

    Returns (t_ns, details). Assumes a warm NEFF cache for the reps=1 program.
    """
    import time
    from concourse.bass_utils import run_bass_kernel_spmd

    in_maps = make_in_maps(source, displacement)

    nc1 = _get_nc()
    ncR = build_bass(reps=reps)
    ncR.finalize()

    run_bass_kernel_spmd(nc1, in_maps, list(range(NCORES)))  # warm compile
    run_bass_kernel_spmd(ncR, in_maps, list(range(NCORES)))

    t1s, tRs = [], []
    for _ in range(warm):
        t0 = time.time(); run_bass_kernel_spmd(nc1, in_maps, list(range(NCORES))); t1s.append(time.time() - t0)
        t0 = time.time(); run_bass_kernel_spmd(ncR, in_maps, list(range(NCORES))); tRs.append(time.time() - t0)
    t1 = min(t1s); tR = min(tRs)
    t_ns = (tR - t1) / (reps - 1) * 1e9
    return t_ns, {"wall_reps": tR, "wall_1": t1, "reps": reps}


if __name__ == "__main__":
    nc = build_bass()
    print("built ok:", len(list(nc.all_instructions())), "instructions")


# revision 31
# speedup vs baseline: 3.1380x; 3.1380x over previous
"""Deformable warp (bilinear grid_sample with shared displacement field) on 8 trn2 cores.

Problem: source [8,16,512,512] f32, displacement [1,2,512,512] f32 (shared over batch).
out[b,c,y,x] = bilinear_sample(source[b,c], x + dx[y,x]*255.5, y + dy[y,x]*255.5),
align_corners=True, zero padding.

Strategy:
  - Host re-layouts source to a channel-last "Z slab": zslab[p] = (px[p], px[p+512])
    where px[p] is pixel p's 128 (b,c) values (512B). One gathered run of 512 f32
    (2KB) starting at zslab[y0*512+x0] therefore contains all four bilinear corners
    (y0/y1 x x0/x1) for one output pixel. The slab is replicated to every core
    (host->HBM upload is not part of HW exec time).
  - Spatial sharding: core q computes output rows [64q, 64q+64) for ALL batches
    and channels. No cross-core communication.
  - Per core: DVE computes sampling coords/weights/indices from the displacement
    rows; GPSIMD indirect DMAs gather one 2KB run per output pixel (one index per
    partition per call - the only indirect-DMA shape the HW DGE supports); DVE
    multiplies by the 4 corner weights (broadcast over the 128 (b,c) lanes along a
    stride-0 AP dim) and reduces; results DMA back to HBM as [pixel, 128] which
    the host transposes back to [B,C,H,W].

Zero-padding semantics are realized by clamping the fetch base into the slab and
zeroing the weights of out-of-image corners (the slab has generous zero padding
so every clamped fetch is in-bounds and finite).
"""

import sys

sys.path.insert(0, "/opt/trn_rl_repo")

import numpy as np

import concourse.bass as bass
import concourse.bacc as bacc
import concourse.mybir as mybir
import concourse.tile as tile

F32 = mybir.dt.float32
I32 = mybir.dt.int32

B, C, H, W = 8, 16, 512, 512
BC = B * C  # 128
NCORES = 8
ROWS = H // NCORES  # 64 output rows per core
NPX = ROWS * W  # 32768 pixels per core
CHUNKS = NPX // 128  # 256 chunks of 128 pixels
TILE_CHUNKS = 16  # chunks per pipeline tile (2048 px)
NTILES = CHUNKS // TILE_CHUNKS  # 16

# Z-slab geometry: ext = [FRONT zero rows][H*W pixel rows][BACK zero rows], each
# row 128 f32.  zslab[i] = ext[i] | ext[i+512], i in [0, len(ext)-512).
# Fetch base for a pixel: i = (y0m*512 + xb) + FRONT with y0m,xb in [-1, 511],
# so min i = FRONT - 513 >= 0 -> FRONT = 513; the run reads zslab[i], zslab[i+1]
# -> ext up to i+1+512: max = FRONT + 262143 + 513 = FRONT + 262656 -> BACK = 514.
FRONT = 513
BACK = 514
NEXT = FRONT + H * W + BACK
NZ = NEXT - 512  # zslab rows

AluOp = mybir.AluOpType


def _ap(handle, offset, dims):
    return bass.AP(handle, offset, [list(d) for d in dims])


def build_bass(reps=1):
    nc = bacc.Bacc()
    nc.num_devices = NCORES

    zslab = nc.declare_dram_parameter("zslab", [NZ, 2 * BC], F32, isOutput=False)
    disp = nc.declare_dram_parameter("disp", [2, ROWS, W], F32, isOutput=False)
    tabs = nc.declare_dram_parameter("tabs", [128, 2 * CHUNKS], F32, isOutput=False)
    out = nc.declare_dram_parameter("out", [NPX, BC], F32, isOutput=True)

    with tile.TileContext(nc) as tc:
        with (
            tc.tile_pool(name="res", bufs=1) as res,
            tc.tile_pool(name="gat", bufs=2) as gat,
            tc.tile_pool(name="ot", bufs=2) as ot,
        ):
            v = nc.vector
            _tagn = [0]

            def rtile(shape, dtype):
                _tagn[0] += 1
                return res.tile(shape, dtype, tag=f"rt{_tagn[0]}", name=f"rt{_tagn[0]}")

            # ---- resident tensors -------------------------------------------------
            # pixel p (raster within this core's 64 rows) lives at
            # [partition = p % 128, chunk = p // 128]; chunk = 4*cy + cx where
            # y_local = cy, x = (p%128) + 128*cx.
            dxy = rtile([128, 2 * CHUNKS], F32)
            wts = rtile([128, CHUNKS, 4], F32)
            idx = rtile([128, CHUNKS], I32)

            # displacement load, both channels in one DMA:
            # value at (part, (ch*ROWS+cy)*4 + cx) = disp[ch, cy, part + 128*cx]
            nc.sync.dma_start(
                out=dxy[:],
                in_=_ap(disp, 0, [(1, 128), (W, 2 * ROWS), (128, 4)]),
            )
            dx = dxy[:, 0:CHUNKS]
            dy = dxy[:, CHUNKS:2 * CHUNKS]

            # per-pixel normalized-coordinate tables (host-arranged, bit-exact
            # jnp.linspace values): xs_pix | ys_pix halves
            tabt = rtile([128, 2 * CHUNKS], F32)
            nc.sync.dma_start(out=tabt[:], in_=tabs[:])
            xs_pix = tabt[:, 0:CHUNKS]
            ys_pix = tabt[:, CHUNKS:2 * CHUNKS]

            # sampling coords in pixel space, matching the reference op-for-op:
            #   g = table + d;  pix = (g + 1) * 0.5 * (size-1)
            gx = rtile([128, CHUNKS], F32)
            gy = rtile([128, CHUNKS], F32)
            v.tensor_tensor(out=gx[:], in0=xs_pix, in1=dx, op=AluOp.add)
            v.tensor_scalar(out=gx[:], in0=gx[:], scalar1=1.0, scalar2=(W - 1) / 2.0,
                            op0=AluOp.add, op1=AluOp.mult)
            v.tensor_tensor(out=gy[:], in0=ys_pix, in1=dy, op=AluOp.add)
            v.tensor_scalar(out=gy[:], in0=gy[:], scalar1=1.0, scalar2=(H - 1) / 2.0,
                            op0=AluOp.add, op1=AluOp.mult)

            def floor_frac(g, lim):
                """returns (g0 = floor(g) f32, frac, w0=1-frac, v0, v1, gb=clamp(g0,-1,lim-1))"""
                t_i = rtile([128, CHUNKS], I32)
                v.tensor_copy(out=t_i[:], in_=g[:])
                tf = rtile([128, CHUNKS], F32)
                v.tensor_copy(out=tf[:], in_=t_i[:])
                adj = rtile([128, CHUNKS], F32)
                v.tensor_tensor(out=adj[:], in0=tf[:], in1=g[:], op=AluOp.is_gt)
                g0 = rtile([128, CHUNKS], F32)
                v.tensor_tensor(out=g0[:], in0=tf[:], in1=adj[:], op=AluOp.subtract)
                fr = rtile([128, CHUNKS], F32)
                v.tensor_tensor(out=fr[:], in0=g[:], in1=g0[:], op=AluOp.subtract)
                w0 = rtile([128, CHUNKS], F32)
                v.tensor_scalar(out=w0[:], in0=fr[:], scalar1=-1.0, scalar2=1.0,
                                op0=AluOp.mult, op1=AluOp.add)
                m0 = rtile([128, CHUNKS], F32)
                m1 = rtile([128, CHUNKS], F32)
                v0 = rtile([128, CHUNKS], F32)
                v1 = rtile([128, CHUNKS], F32)
                v.tensor_scalar(out=m0[:], in0=g0[:], scalar1=0.0, scalar2=None, op0=AluOp.is_ge)
                v.tensor_scalar(out=m1[:], in0=g0[:], scalar1=float(lim - 1), scalar2=None, op0=AluOp.is_le)
                v.tensor_tensor(out=v0[:], in0=m0[:], in1=m1[:], op=AluOp.mult)
                v.tensor_scalar(out=m0[:], in0=g0[:], scalar1=-1.0, scalar2=None, op0=AluOp.is_ge)
                v.tensor_scalar(out=m1[:], in0=g0[:], scalar1=float(lim - 2), scalar2=None, op0=AluOp.is_le)
                v.tensor_tensor(out=v1[:], in0=m0[:], in1=m1[:], op=AluOp.mult)
                gb = rtile([128, CHUNKS], F32)
                v.tensor_scalar(out=gb[:], in0=g0[:], scalar1=-1.0, scalar2=float(lim - 1),
                                op0=AluOp.max, op1=AluOp.min)
                return g0, fr, w0, v0, v1, gb

            x0f, fx, wx0, vx0, vx1, xb = floor_frac(gx, W)
            y0f, fy, wy0, vy0, vy1, yb = floor_frac(gy, H)

            # gather index first (unblocks the gather pipeline):
            # (yb*512 + xb) + FRONT, all values exact in f32
            idf = rtile([128, CHUNKS], F32)
            v.scalar_tensor_tensor(out=idf[:], in0=yb[:], scalar=float(W), in1=xb[:],
                                   op0=AluOp.mult, op1=AluOp.add)
            v.tensor_scalar(out=idf[:], in0=idf[:], scalar1=float(FRONT), scalar2=None,
                            op0=AluOp.add)
            v.tensor_copy(out=idx[:], in_=idf[:])

            # masked 1-D weights
            wxa = rtile([128, CHUNKS], F32)
            wxb = rtile([128, CHUNKS], F32)
            wya = rtile([128, CHUNKS], F32)
            wyb = rtile([128, CHUNKS], F32)
            v.tensor_tensor(out=wxa[:], in0=wx0[:], in1=vx0[:], op=AluOp.mult)
            v.tensor_tensor(out=wxb[:], in0=fx[:], in1=vx1[:], op=AluOp.mult)
            v.tensor_tensor(out=wya[:], in0=wy0[:], in1=vy0[:], op=AluOp.mult)
            v.tensor_tensor(out=wyb[:], in0=fy[:], in1=vy1[:], op=AluOp.mult)

            # corner weights, gathered-run order (r0x0, r1x0, r0x1, r1x1):
            for k, (a, b) in enumerate(((wya, wxa), (wyb, wxa), (wya, wxb), (wyb, wxb))):
                wk = _ap(wts.tensor, wts[:].offset + k, [(wts[:].ap[0][0], 128), (4, CHUNKS)])
                v.tensor_tensor(out=wk, in0=a[:], in1=b[:], op=AluOp.mult)

            # ---- main pipeline ----------------------------------------------------
            import contextlib
            loop_ctx = tc.For_i(0, reps) if reps > 1 else contextlib.nullcontext()
            with loop_ctx:
                main_pipeline(nc, tc, v, zslab, out, wts, idx, gat, ot)

    return nc


def main_pipeline(nc, tc, v, zslab, out, wts, idx, gat, ot):
    if True:
        if True:
            for t in range(NTILES):
                # gathered tile: memory [part][chunk][4 corners x 128bc]
                g = gat.tile([128, TILE_CHUNKS, 4 * BC], F32)
                gp = g[:].ap[0][0]
                for c in range(TILE_CHUNKS):
                    cg = t * TILE_CHUNKS + c
                    nc.gpsimd.indirect_dma_start(
                        out=g[:, c, :],
                        out_offset=None,
                        in_=zslab[:],
                        in_offset=bass.IndirectOffsetOnAxis(ap=idx[:, cg:cg + 1], axis=0),
                    )

                # multiply by corner weights (broadcast over the 128 bc lanes);
                # note: the DVE runs stride-0 broadcasts well below line rate, but
                # this ordering (contiguous innermost on G/out) measured best.
                g_m = _ap(g.tensor, g[:].offset,
                          [(gp, 128), (4 * BC, TILE_CHUNKS), (BC, 4), (1, BC)])
                w_m = _ap(wts.tensor, wts[:].offset + t * TILE_CHUNKS * 4,
                          [(wts[:].ap[0][0], 128), (4, TILE_CHUNKS), (1, 4), (0, BC)])
                v.tensor_tensor(out=g_m, in0=g_m, in1=w_m, op=AluOp.mult)

                # reduce the 4 corners: pairwise adds (cheaper than tensor_reduce:
                # each 2-input add reads both operands in one cycle)
                h = ot.tile([128, TILE_CHUNKS, 2 * BC], F32, tag="h", name=f"h_{t}")
                ga = _ap(g.tensor, g[:].offset,
                         [(gp, 128), (4 * BC, TILE_CHUNKS), (1, 2 * BC)])
                gb2 = _ap(g.tensor, g[:].offset + 2 * BC,
                          [(gp, 128), (4 * BC, TILE_CHUNKS), (1, 2 * BC)])
                v.tensor_tensor(out=h[:], in0=ga, in1=gb2, op=AluOp.add)
                o = ot.tile([128, TILE_CHUNKS, BC], F32)
                hp = h[:].ap[0][0]
                ha = _ap(h.tensor, h[:].offset,
                         [(hp, 128), (2 * BC, TILE_CHUNKS), (1, BC)])
                hb = _ap(h.tensor, h[:].offset + BC,
                         [(hp, 128), (2 * BC, TILE_CHUNKS), (1, BC)])
                v.tensor_tensor(out=o[:], in0=ha, in1=hb, op=AluOp.add)

                # writeback: pixel p = part + 128*(t*TILE_CHUNKS + chunk) at out[p, :]
                out_t = _ap(out, t * TILE_CHUNKS * 128 * BC,
                            [(BC, 128), (128 * BC, TILE_CHUNKS), (1, BC)])
                nc.sync.dma_start(out=out_t, in_=o[:])


def linspace_tables():
    """The reference's jnp.linspace(-1, 1, size) values, bit-exact (computed on CPU)."""
    import jax

    with jax.default_device(jax.devices("cpu")[0]):
        xs = np.asarray(jax.numpy.linspace(-1.0, 1.0, W, dtype=np.float32))
        ys = np.asarray(jax.numpy.linspace(-1.0, 1.0, H, dtype=np.float32))
    return xs, ys


def coord_tables(q):
    """Per-pixel linspace tables in the kernel's [part, chunk] pixel layout, core q.
    Returns one [128, 2*CHUNKS] array: xs half | ys half."""
    xs, ys = linspace_tables()
    cx = np.arange(CHUNKS) % 4
    cy = np.arange(CHUNKS) // 4
    part = np.arange(128)
    xs_pix = xs[part[:, None] + 128 * cx[None, :]]
    ys_pix = np.broadcast_to(ys[q * ROWS + cy][None, :], (128, CHUNKS))
    return np.ascontiguousarray(np.hstack([xs_pix, ys_pix]), np.float32)


def build_zslab(source):
    """Channel-last Z slab: zslab[i] = ext[i] | ext[i+512]."""
    ext = np.zeros((NEXT, BC), np.float32)
    ext[FRONT:FRONT + H * W] = source.transpose(2, 3, 0, 1).reshape(H * W, BC)
    z = np.empty((NZ, 2 * BC), np.float32)
    z[:, :BC] = ext[:NZ]
    z[:, BC:] = ext[512:512 + NZ]
    return z


def make_in_maps(source, displacement):
    source = np.ascontiguousarray(source, dtype=np.float32)
    displacement = np.ascontiguousarray(displacement, dtype=np.float32)
    assert source.shape == (B, C, H, W)
    assert displacement.shape == (1, 2, H, W)
    z = build_zslab(source)
    d = displacement[0]
    in_maps = []
    for q in range(NCORES):
        in_maps.append({
            "zslab": z,
            "disp": np.ascontiguousarray(d[:, q * ROWS:(q + 1) * ROWS, :]),
            "tabs": coord_tables(q),
        })
    return in_maps


_NC_CACHE = None


def _get_nc():
    global _NC_CACHE
    if _NC_CACHE is None:
        _NC_CACHE = build_bass()
        if not _NC_CACHE.is_finalized():
            _NC_CACHE.finalize()
    return _NC_CACHE


def assemble_output(outs):
    full = np.concatenate([o.reshape(ROWS, W, B, C) for o in outs], axis=0)
    return np.ascontiguousarray(full.transpose(2, 3, 0, 1))


def kernel(source, displacement):
    from concourse.bass_utils import run_bass_kernel_spmd

    in_maps = make_in_maps(source, displacement)
    res = run_bass_kernel_spmd(_get_nc(), in_maps, list(range(NCORES)))
    return assemble_output([res.results[q]["out"] for q in range(NCORES)])


def measure_hw(source, displacement, reps=4097, warm=2):
    """Estimate per-invocation HW time via a device-looped program.

    Returns (t_ns, details). Assumes a warm NEFF cache for the reps=1 program.
    """
    import time
    from concourse.bass_utils import run_bass_kernel_spmd

    in_maps = make_in_maps(source, displacement)

    nc1 = _get_nc()
    ncR = build_bass(reps=reps)
    ncR.finalize()

    run_bass_kernel_spmd(nc1, in_maps, list(range(NCORES)))  # warm compile
    run_bass_kernel_spmd(ncR, in_maps, list(range(NCORES)))

    t1s, tRs = [], []
    for _ in range(warm):
        t0 = time.time(); run_bass_kernel_spmd(nc1, in_maps, list(range(NCORES))); t1s.append(time.time() - t0)
        t0 = time.time(); run_bass_kernel_spmd(ncR, in_maps, list(range(NCORES))); tRs.append(time.time() - t0)
    t1 = min(t1s); tR = min(tRs)
    t_ns = (tR - t1) / (reps - 1) * 1e9
    return t_ns, {"wall_reps": tR, "wall_1": t1, "reps": reps}


if __name__ == "__main__":
    nc = build_bass()
    print("built ok:", len(list(nc.all_instructions())), "instructions")
